# revision 1
# baseline (speedup 1.0000x reference)
"""Fused LayerNorm + causal multi-head attention + output projection for
Trainium2, distributed over 8 NeuronCores.

Problem (full shapes): x [4, 2048, 1024], g_ln [1024], Wq [1024, 1024],
Wkv [1024, 2048], Wo [1024, 1024]; B=4, N=2048, D=1024, H=16, DH=64.

Sharding: DP(batch)=4 x TP(heads)=2. Core c handles batch b=c//2 and head
group g=c%2 (heads [g*8, g*8+8)). Each core computes LN(x_b), projects
q/k/v for its 8 heads (g_ln and the 1/sqrt(DH) scale are folded into the
weights host-side), runs causal attention, and multiplies by its slice of
Wo rows, producing a partial [2048, 1024] output. The host sums the two
partials per batch (row-parallel Wo reduce done on host).

On-chip layout notes:
 - All matmuls run as float32r (full-rate fp32 path on the PE).
 - Scores are computed transposed (S^T[j, i]) so softmax denominators come
   from a ones-column appended to V, and no P transposes are needed.
 - Causal masking multiplies the post-exp diagonal [128,128] block by a
   binary lower-triangle; fully-masked columns left of the diagonal are
   skipped entirely (trimmed QK/exp/PV ranges).
"""

import sys

for _p in ("/opt/trn_rl_repo",):
    if _p not in sys.path:
        sys.path.insert(0, _p)

import numpy as np

import concourse.bacc as bacc
import concourse.mybir as mybir
import concourse.tile as tile
from concourse.bass_utils import run_bass_kernel_spmd

N_CORES = 8
B, N, D, H = 4, 2048, 1024, 16
DH = D // H
HL = 8  # heads per core
EPS = 1e-5
F32 = mybir.dt.float32
F32R = mybir.dt.float32r


def build_module(repeat: int = 1):
    nc = bacc.Bacc("TRN2", target_bir_lowering=False)

    x_h = nc.dram_tensor("x", [N, D], F32, kind="ExternalInput")
    wq_h = nc.dram_tensor("wq", [D, 512], F32R, kind="ExternalInput")
    wk_h = nc.dram_tensor("wk", [D, 512], F32R, kind="ExternalInput")
    wv_h = nc.dram_tensor("wv", [D, 512], F32R, kind="ExternalInput")
    wo_h = nc.dram_tensor("wo", [512, D], F32R, kind="ExternalInput")
    tri_h = nc.dram_tensor("tri", [128, 128], F32, kind="ExternalInput")
    ident_h = nc.dram_tensor("ident", [128, 128], F32, kind="ExternalInput")
    out_h = nc.dram_tensor("out", [N, D], F32, kind="ExternalOutput")

    with tile.TileContext(nc) as tc:

        def body(_iv=None):
            _body(nc, tc, x_h, wq_h, wk_h, wv_h, wo_h, tri_h, ident_h, out_h)

        if repeat == 1:
            body()
        else:
            with tc.For_i(0, repeat, 1):
                body()

    nc.compile()
    return nc


def _body(nc, tc, x_h, wq_h, wk_h, wv_h, wo_h, tri_h, ident_h, out_h):
    from contextlib import ExitStack

    with ExitStack() as ctx:
        persist = ctx.enter_context(tc.tile_pool(name="persist", bufs=1))

        identsb = persist.tile([128, 128], F32)
        nc.sync.dma_start(out=identsb, in_=ident_h[:, :])
        trisb = persist.tile([128, 128], F32)
        nc.sync.dma_start(out=trisb, in_=tri_h[:, :])

        qT = persist.tile([128, 4, N], F32R)
        kT = persist.tile([128, 4, N], F32R)
        vsc = persist.tile([128, 16, HL, 65], F32R)
        OTsb = persist.tile([128, 4, N], F32R)

        # ---------------- Phase A+B: LayerNorm -> xn^T, projections -------
        with ExitStack() as ab:
            abp = ab.enter_context(tc.tile_pool(name="abp", bufs=1))
            lnp = ab.enter_context(tc.tile_pool(name="lnp", bufs=2))
            wsp = ab.enter_context(tc.tile_pool(name="wsp", bufs=2))
            psA = ab.enter_context(tc.tile_pool(name="psA", bufs=3, space="PSUM"))
            psB = ab.enter_context(tc.tile_pool(name="psB", bufs=2, space="PSUM"))

            eps_t = abp.tile([128, 1], F32)
            nc.vector.memset(eps_t, EPS)
            ones8 = abp.tile([128, 8], F32)
            nc.vector.memset(ones8, 1.0)

            wv_sb = abp.tile([128, 8, 512], F32R)

            xnT_q = [None] * 4
            for th in range(4):  # token quarters, double-buffered xnT
                t0 = th * 512
                xnT = abp.tile([128, 8, 512], F32R, tag="xnT", bufs=2)
                xnT_q[th] = xnT

                for tt in range(4):
                    xt = lnp.tile([128, D], F32, tag="xt", bufs=3)
                    nc.sync.dma_start(
                        out=xt, in_=x_h[t0 + tt * 128 : t0 + (tt + 1) * 128, :]
                    )
                    st = lnp.tile([128, 2, 6], F32, tag="st")
                    for sg in range(2):
                        nc.vector.bn_stats(
                            out=st[:, sg, :], in_=xt[:, sg * 512 : (sg + 1) * 512]
                        )
                    mv = lnp.tile([128, 2], F32, tag="mv")
                    nc.vector.bn_aggr(out=mv, in_=st)
                    rs = lnp.tile([128, 1], F32, tag="rs")
                    nc.scalar.activation(
                        out=rs, in_=mv[:, 1:2],
                        func=mybir.ActivationFunctionType.Sqrt,
                        bias=eps_t, scale=1.0,
                    )
                    nc.vector.reciprocal(out=rs, in_=rs)
                    nc.vector.tensor_scalar(
                        out=xt, in0=xt, scalar1=mv[:, 0:1], scalar2=rs,
                        op0=mybir.AluOpType.subtract, op1=mybir.AluOpType.mult,
                    )
                    for grp in range(2):
                        trp = psA.tile([128, 4, 128], F32, tag="trp")
                        for j in range(4):
                            dk = grp * 4 + j
                            nc.tensor.matmul(
                                trp[:, j, :],
                                lhsT=xt[:, dk * 128 : (dk + 1) * 128],
                                rhs=identsb, is_transpose=True,
                                start=True, stop=True,
                            )
                        nc.scalar.copy(
                            out=xnT[:, grp * 4 : grp * 4 + 4, tt * 128 : (tt + 1) * 128],
                            in_=trp,
                        )

                if th == 0:
                    nc.sync.dma_start(
                        out=wv_sb,
                        in_=wv_h[:, :].rearrange("(dk r) m -> r dk m", r=128),
                    )

                # v projection for this quarter
                for tt in range(4):
                    psv = psB.tile([128, 512], F32, tag="psv")
                    for dk in range(8):
                        nc.tensor.matmul(
                            psv, lhsT=xnT[:, dk, tt * 128 : (tt + 1) * 128],
                            rhs=wv_sb[:, dk, :],
                            start=(dk == 0), stop=(dk == 7),
                        )
                    jt = th * 4 + tt
                    nc.vector.tensor_copy(
                        out=vsc[:, jt, :, 0:64],
                        in_=psv.rearrange("r (h d) -> r h d", h=HL),
                    )
                    nc.vector.tensor_copy(
                        out=vsc[:, jt, :, 64:65].rearrange("p h o -> p (h o)"),
                        in_=ones8,
                    )

                # q/k projections once per half, reading both live quarters
                if th % 2 == 1:
                    h0 = (th - 1) * 512
                    for p in range(4):
                        wqs = wsp.tile([128, 8, 128], F32R, tag="wqs")
                        nc.sync.dma_start(
                            out=wqs,
                            in_=wq_h[:, p * 128 : (p + 1) * 128].rearrange(
                                "(dk r) m -> r dk m", r=128
                            ),
                        )
                        wks = wsp.tile([128, 8, 128], F32R, tag="wks")
                        nc.sync.dma_start(
                            out=wks,
                            in_=wk_h[:, p * 128 : (p + 1) * 128].rearrange(
                                "(dk r) m -> r dk m", r=128
                            ),
                        )
                        for t4 in range(2):
                            xq = xnT_q[th - 1 + t4]
                            psq = psB.tile([128, 512], F32, tag="pqk")
                            for dk in range(8):
                                nc.tensor.matmul(
                                    psq, lhsT=wqs[:, dk, :],
                                    rhs=xq[:, dk, :],
                                    start=(dk == 0), stop=(dk == 7),
                                )
                            nc.scalar.copy(
                                out=qT[:, p, h0 + t4 * 512 : h0 + (t4 + 1) * 512],
                                in_=psq,
                            )
                            psk = psB.tile([128, 512], F32, tag="pqk")
                            for dk in range(8):
                                nc.tensor.matmul(
                                    psk, lhsT=wks[:, dk, :],
                                    rhs=xq[:, dk, :],
                                    start=(dk == 0), stop=(dk == 7),
                                )
                            nc.scalar.copy(
                                out=kT[:, p, h0 + t4 * 512 : h0 + (t4 + 1) * 512],
                                in_=psk,
                            )

        # ---- Phase C/D interleaved: attention per i-half, then that
        # half's output projection (psS 4 + psO 2 + psD 2 = 8 banks) -------
        dsp = ctx.enter_context(tc.tile_pool(name="dsp", bufs=1))
        outp = ctx.enter_context(tc.tile_pool(name="outp", bufs=4))
        psD = ctx.enter_context(tc.tile_pool(name="psD", bufs=2, space="PSUM"))
        wo_sb = dsp.tile([128, 4, D], F32R)
        nc.sync.dma_start(
            out=wo_sb, in_=wo_h[:, :].rearrange("(ck r) e -> r ck e", r=128)
        )

        def wo_half(ihalf, pool):
            for tt in range(ihalf * 8, ihalf * 8 + 8):
                for e2 in range(2):
                    pso = pool.tile([128, 512], F32, tag="pso")
                    for ck in range(4):
                        nc.tensor.matmul(
                            pso, lhsT=OTsb[:, ck, tt * 128 : (tt + 1) * 128],
                            rhs=wo_sb[:, ck, e2 * 512 : (e2 + 1) * 512],
                            start=(ck == 0), stop=(ck == 3),
                        )
                    osb = outp.tile([128, 512], F32, tag="osb")
                    nc.vector.tensor_copy(out=osb, in_=pso)
                    nc.sync.dma_start(
                        out=out_h[tt * 128 : (tt + 1) * 128,
                                  e2 * 512 : (e2 + 1) * 512],
                        in_=osb,
                    )

        with ExitStack() as cs:
            expp = cs.enter_context(tc.tile_pool(name="expp", bufs=3))
            denp = cs.enter_context(tc.tile_pool(name="denp", bufs=3))
            drp = cs.enter_context(tc.tile_pool(name="drp", bufs=2, space="DRAM"))
            psS = cs.enter_context(tc.tile_pool(name="psS", bufs=2, space="PSUM"))
            psO = cs.enter_context(tc.tile_pool(name="psO", bufs=1, space="PSUM"))

            for ihalf in range(2):
                half0 = ihalf * 1024
                for p in range(4):
                    for hh in range(2):
                        h = p * 2 + hh
                        row0 = hh * 64
                        OTp = psO.tile([128, 1024], F32, tag="OTp")
                        for ji in range(8 if ihalf == 0 else 16):
                            dt_i = (ji * 128) // 512 * 512
                            i_lo = max(half0, dt_i)
                            W = half0 + 1024 - i_lo
                            nblk = W // 512
                            d = 0
                            has_mask = dt_i >= half0
                            if has_mask:
                                d = ji * 128 - dt_i
                            Sp = psS.tile([128, 1024], F32, tag="Sp")
                            for s5 in range(nblk):
                                lo = d if s5 == 0 else 0
                                nc.tensor.matmul(
                                    Sp[:, s5 * 512 + lo : (s5 + 1) * 512],
                                    lhsT=kT[row0 : row0 + 64, p,
                                            ji * 128 : (ji + 1) * 128],
                                    rhs=qT[row0 : row0 + 64, p,
                                           i_lo + s5 * 512 + lo : i_lo + (s5 + 1) * 512],
                                    start=True, stop=True,
                                )
                            expS = expp.tile([128, 1024], F32R, tag="expS")
                            nc.scalar.activation(
                                out=expS[:, d:W], in_=Sp[:, d:W],
                                func=mybir.ActivationFunctionType.Exp,
                            )
                            if has_mask:
                                nc.vector.tensor_tensor(
                                    out=expS[:, d : d + 128],
                                    in0=expS[:, d : d + 128], in1=trisb,
                                    op=mybir.AluOpType.mult,
                                )
                            for s5 in range(nblk):
                                blk_i = i_lo + s5 * 512
                                off = blk_i - half0
                                lo = d if s5 == 0 else 0
                                nc.tensor.matmul(
                                    OTp[0:65, off + lo : off + 512],
                                    lhsT=vsc[:, ji, h, :],
                                    rhs=expS[:, s5 * 512 + lo : (s5 + 1) * 512],
                                    start=(ji == 0), stop=(ji == blk_i // 128 + 3),
                                )
                        # copy out early (releases the OT psum slot), then
                        # normalize rows 0..63 by 1/rowsum (row 64).
                        cp = denp.tile([128, 1024], F32, tag="cp")
                        nc.vector.tensor_copy(out=cp[0:65, :], in_=OTp[0:65, :])
                        nc.vector.reciprocal(out=cp[64:65, :], in_=cp[64:65, :])
                        dscr = drp.tile([1, 1024], F32, tag="dscr")
                        nc.sync.dma_start(out=dscr, in_=cp[64:65, :])
                        bc = denp.tile([128, 1024], F32, tag="bc")
                        nc.sync.dma_start(
                            out=bc[0:64, :], in_=dscr.broadcast_to((64, 1024))
                        )
                        nc.vector.tensor_tensor(
                            out=cp[0:64, :], in0=cp[0:64, :], in1=bc[0:64, :],
                            op=mybir.AluOpType.mult,
                        )
                        nc.sync.dma_start(
                            out=OTsb[row0 : row0 + 64, p, half0 : half0 + 1024],
                            in_=cp[0:64, :].bitcast(F32R),
                        )

                if ihalf == 0:
                    wo_half(0, psD)

        # attention pools closed: 6 banks free for a deep final Wo pipeline
        psD2 = ctx.enter_context(tc.tile_pool(name="psD2", bufs=6, space="PSUM"))
        wo_half(1, psD2)


_CACHE = {}


def _get_module(repeat: int = 1):
    if repeat not in _CACHE:
        _CACHE[repeat] = build_module(repeat)
    return _CACHE[repeat]


def _make_tri():
    r = np.arange(128)[:, None]
    c = np.arange(128)[None, :]
    return (c >= r).astype(np.float32)  # 1 = attend (j <= i), 0 = masked


def _prep_in_maps(x, g_ln, Wq, Wkv, Wo):
    x = np.asarray(x, dtype=np.float32)
    g_ln = np.asarray(g_ln, dtype=np.float32)
    Wq = np.asarray(Wq, dtype=np.float32)
    Wkv = np.asarray(Wkv, dtype=np.float32)
    Wo = np.asarray(Wo, dtype=np.float32)

    scale = np.float32(DH ** -0.5)
    wq_full = (g_ln[:, None] * Wq * scale).astype(np.float32)
    wk_full = (g_ln[:, None] * Wkv[:, :D]).astype(np.float32)
    wv_full = (g_ln[:, None] * Wkv[:, D:]).astype(np.float32)

    tri = _make_tri()
    ident = np.eye(128, dtype=np.float32)

    in_maps = []
    for c in range(N_CORES):
        b, g = c // 2, c % 2
        sl = slice(g * 512, (g + 1) * 512)
        in_maps.append(
            {
                "x": np.ascontiguousarray(x[b]),
                "wq": np.ascontiguousarray(wq_full[:, sl]),
                "wk": np.ascontiguousarray(wk_full[:, sl]),
                "wv": np.ascontiguousarray(wv_full[:, sl]),
                "wo": np.ascontiguousarray(Wo[sl, :]),
                "tri": tri,
                "ident": ident,
            }
        )
    return in_maps


def kernel(x, g_ln, Wq, Wkv, Wo):
    nc = _get_module(repeat=1)
    in_maps = _prep_in_maps(x, g_ln, Wq, Wkv, Wo)
    res = run_bass_kernel_spmd(nc, in_maps, list(range(N_CORES)))
    out = np.empty((B, N, D), dtype=np.float32)
    for b in range(B):
        out[b] = res.results[2 * b]["out"] + res.results[2 * b + 1]["out"]
    return out



# revision 32
# speedup vs baseline: 1.4936x; 1.4936x over previous
"""Fused LayerNorm + causal multi-head attention + output projection for
Trainium2, distributed over 8 NeuronCores.

Problem (full shapes): x [4, 2048, 1024], g_ln [1024], Wq [1024, 1024],
Wkv [1024, 2048], Wo [1024, 1024]; B=4, N=2048, D=1024, H=16, DH=64.

Sharding: DP(batch)=4 x TP(heads)=2. Core c handles batch b=c//2 and head
group g=c%2 (heads [g*8, g*8+8)). Each core computes LN(x_b), projects
q/k/v for its 8 heads (g_ln and the 1/sqrt(DH) scale are folded into the
weights host-side), runs causal attention, and multiplies by its slice of
Wo rows, producing a partial [2048, 1024] output. The host sums the two
partials per batch (row-parallel Wo reduce done on host).

v2 design notes (vs the fp32r v1):
 - All activations/weights are bf16 on chip (matmuls run at the same
   1 cycle/row as fp32r, but transposes are 2x faster and SBUF/DMA
   traffic halves). PSUM accumulation stays fp32.
 - xn^T is produced by the DMA XBAR transpose (16-bit only), freeing
   both the PE (no transpose matmuls) and ACT (no PSUM->SBUF copies).
 - Scores are computed transposed (S^T[j, i]) so softmax denominators
   come from a ones-column appended to V; no P transposes needed.
 - Causal masking multiplies the post-exp diagonal [128,128] block by a
   binary lower-triangle; fully-masked blocks are skipped (trimmed
   QK/exp/PV ranges).
 - Emission order software-pipelines the phases: LN/proj of the second
   token half and the i<1024 attention triangle interleave, and the
   Wo projection of the first half streams inside the i>=1024 attention
   window, keeping PE busy while ACT grinds exp().
"""

import sys

for _p in ("/opt/trn_rl_repo",):
    if _p not in sys.path:
        sys.path.insert(0, _p)

import numpy as np
import ml_dtypes

import concourse.bacc as bacc
import concourse.mybir as mybir
import concourse.tile as tile
from concourse.bass_utils import run_bass_kernel_spmd

# Route every ACT function this kernel uses (Ln, Exp, Copy) to the single
# table set that contains them all, so insert_act_table_loads emits exactly
# one ACT_TABLE_LOAD instead of thrashing between per-function sets (the
# pass picks the first matching set otherwise: Exp->exp_and_others,
# Ln->natural_log => a 1.3us reload per LN tile interleaved with attention
# exp). Indices must stay stable (walrus keys act.json by list position),
# so the other entries are kept but emptied rather than removed.
import concourse.hw_specs as _hw_specs
import concourse.bacc as _bacc_mod
import functools as _functools

_orig_get_act_tables = _hw_specs.get_activation_tables


@_functools.cache
def _patched_get_act_tables(arch):
    tabs = dict(_orig_get_act_tables(arch))
    keep = "natural_log_exp_and_others"
    assert keep in tabs
    need = {
        mybir.ActivationFunctionType.Ln,
        mybir.ActivationFunctionType.Exp,
        mybir.ActivationFunctionType.Copy,
        mybir.ActivationFunctionType.Identity,
    }
    assert need <= tabs[keep], (need, tabs[keep])
    return {name: (fs if name == keep else set()) for name, fs in tabs.items()}


_hw_specs.get_activation_tables = _patched_get_act_tables
_bacc_mod.get_activation_tables = _patched_get_act_tables
import concourse.bass_interp as _bass_interp_mod

_bass_interp_mod.get_activation_tables = _patched_get_act_tables

N_CORES = 8
B, N, D, H = 4, 2048, 1024, 16
DH = D // H
HL = 8  # heads per core
EPS = 1e-5
F32 = mybir.dt.float32
BF16 = mybir.dt.bfloat16


def build_module(repeat: int = 1):
    nc = bacc.Bacc("TRN2", target_bir_lowering=False)

    x_h = nc.dram_tensor("x", [N, D], BF16, kind="ExternalInput")
    # weights come host-prelayouted as [r, dk, m] so the DMA reads one
    # contiguous 8KB run per partition (128 descriptors, not 1024)
    wq_h = nc.dram_tensor("wq", [128, 8, 512], BF16, kind="ExternalInput")
    wk_h = nc.dram_tensor("wk", [128, 8, 512], BF16, kind="ExternalInput")
    wv_h = nc.dram_tensor("wv", [128, 8, 512], BF16, kind="ExternalInput")
    wo_h = nc.dram_tensor("wo", [128, 4, D], BF16, kind="ExternalInput")
    tri_h = nc.dram_tensor("tri", [128, 128], BF16, kind="ExternalInput")
    out_h = nc.dram_tensor("out", [N, D], BF16, kind="ExternalOutput")

    with tile.TileContext(nc) as tc:

        def body(_iv=None):
            _body(nc, tc, x_h, wq_h, wk_h, wv_h, wo_h, tri_h, out_h)

        if repeat == 1:
            body()
        else:
            with tc.For_i(0, repeat, 1):
                body()

    nc.compile()
    return nc


def _body(nc, tc, x_h, wq_h, wk_h, wv_h, wo_h, tri_h, out_h):
    from contextlib import ExitStack

    with ExitStack() as ctx:
        persist = ctx.enter_context(tc.tile_pool(name="persist", bufs=1))

        trisb = persist.tile([128, 128], BF16)
        nc.sync.dma_start(out=trisb, in_=tri_h[:, :])

        xnT = persist.tile([128, 8, N], BF16)
        qT = persist.tile([128, 4, N], BF16)
        kT = persist.tile([128, 4, N], BF16)
        vsc = persist.tile([128, 16, HL, 65], BF16)
        OTsb = persist.tile([128, 4, N], BF16)

        wv_sb = persist.tile([128, 8, 512], BF16)
        wo_sb = persist.tile([128, 4, D], BF16)
        wq_sb = persist.tile([128, 8, 512], BF16)
        wk_sb = persist.tile([128, 8, 512], BF16)

        abp = ctx.enter_context(tc.tile_pool(name="abp", bufs=1))
        lnp = ctx.enter_context(tc.tile_pool(name="lnp", bufs=3))
        wsp = ctx.enter_context(tc.tile_pool(name="wsp", bufs=2))

        eps_t = abp.tile([128, 1], F32)
        nc.vector.memset(eps_t, EPS)
        ones8 = abp.tile([128, 8], BF16)
        nc.vector.memset(ones8, 1.0)

        # ---------------- helpers ----------------------------------------
        def ln_tile(tt):
            """LN token tile tt -> xnT[:, :, tt*128:(tt+1)*128] (bf16)."""
            t0 = tt * 128
            xt = lnp.tile([128, D], BF16, tag="xt", bufs=4)
            nc.sync.dma_start(out=xt, in_=x_h[t0 : t0 + 128, :])
            st = lnp.tile([128, 2, 6], F32, tag="st")
            for sg in range(2):
                nc.vector.bn_stats(
                    out=st[:, sg, :], in_=xt[:, sg * 512 : (sg + 1) * 512]
                )
            mv = lnp.tile([128, 2], F32, tag="mv")
            nc.vector.bn_aggr(out=mv, in_=st)
            # rsqrt(var+eps) = exp(-0.5*ln(var+eps)); ln+exp live in one ACT
            # table set (natural_log_exp_and_others) so interleaving with the
            # attention exp() does not thrash ACT_TABLE_LOAD.
            lg = lnp.tile([128, 1], F32, tag="lg")
            nc.scalar.activation(
                out=lg, in_=mv[:, 1:2],
                func=mybir.ActivationFunctionType.Ln,
                bias=eps_t, scale=1.0,
            )
            rs = lnp.tile([128, 1], F32, tag="rs")
            nc.scalar.activation(
                out=rs, in_=lg,
                func=mybir.ActivationFunctionType.Exp,
                scale=-0.5,
            )
            xtn = lnp.tile([128, D], BF16, tag="xtn", bufs=3)
            nc.vector.tensor_scalar(
                out=xtn, in0=xt, scalar1=mv[:, 0:1], scalar2=rs,
                op0=mybir.AluOpType.subtract, op1=mybir.AluOpType.mult,
            )
            # XBAR transpose: [128 tok, 1024 d] -> [128 r, 8 dk, 128 tok]
            nc.sync.dma_start(
                out=xnT[:, :, t0 : t0 + 128], in_=xtn, transpose=True
            )

        def load_wv():
            nc.sync.dma_start(out=wv_sb, in_=wv_h[:, :, :])

        def load_wo():
            nc.sync.dma_start(out=wo_sb, in_=wo_h[:, :, :])

        def load_wqk():
            nc.sync.dma_start(out=wq_sb, in_=wq_h[:, :, :])
            nc.sync.dma_start(out=wk_sb, in_=wk_h[:, :, :])

        def vproj(tt, pool):
            """v projection for token tile tt -> vsc[:, tt, :, :]."""
            psv = pool.tile([128, 512], F32, tag="pp")
            for dk in range(8):
                nc.tensor.matmul(
                    psv, lhsT=xnT[:, dk, tt * 128 : (tt + 1) * 128],
                    rhs=wv_sb[:, dk, :],
                    start=(dk == 0), stop=(dk == 7),
                )
            nc.vector.tensor_copy(
                out=vsc[:, tt, :, 0:64],
                in_=psv.rearrange("r (h d) -> r h d", h=HL),
            )
            nc.vector.tensor_copy(
                out=vsc[:, tt, :, 64:65].rearrange("p h o -> p (h o)"),
                in_=ones8,
            )

        def qkproj(p, t4, pool):
            """q/k projection for dim block p, token half-quarter t4 (512)."""
            p0 = p * 128
            tok0 = t4 * 512
            psq = pool.tile([128, 512], F32, tag="pp")
            for dk in range(8):
                nc.tensor.matmul(
                    psq, lhsT=wq_sb[:, dk, p0 : p0 + 128],
                    rhs=xnT[:, dk, tok0 : tok0 + 512],
                    start=(dk == 0), stop=(dk == 7),
                )
            nc.vector.tensor_copy(out=qT[:, p, tok0 : tok0 + 512], in_=psq)
            psk = pool.tile([128, 512], F32, tag="pp")
            for dk in range(8):
                nc.tensor.matmul(
                    psk, lhsT=wk_sb[:, dk, p0 : p0 + 128],
                    rhs=xnT[:, dk, tok0 : tok0 + 512],
                    start=(dk == 0), stop=(dk == 7),
                )
            nc.vector.tensor_copy(out=kT[:, p, tok0 : tok0 + 512], in_=psk)

        def wo_tile(tt, e2, pool):
            pso = pool.tile([128, 512], F32, tag="pso")
            for ck in range(4):
                nc.tensor.matmul(
                    pso, lhsT=OTsb[:, ck, tt * 128 : (tt + 1) * 128],
                    rhs=wo_sb[:, ck, e2 * 512 : (e2 + 1) * 512],
                    start=(ck == 0), stop=(ck == 3),
                )
            osb = outp.tile([128, 512], BF16, tag="osb")
            nc.vector.tensor_copy(out=osb, in_=pso)
            nc.sync.dma_start(
                out=out_h[tt * 128 : (tt + 1) * 128, e2 * 512 : (e2 + 1) * 512],
                in_=osb,
            )

        outp = ctx.enter_context(tc.tile_pool(name="outp", bufs=4))

        def attn_head(ihalf, p, hh, fill=None):
            """Causal attention for head (p, hh), query half ihalf.

            fill: optional list of zero-arg closures; one is invoked after
            each ji iteration to interleave non-attention work into the
            emission stream (software pipelining across engines).
            """
            half0 = ihalf * 1024
            row0 = hh * 64
            h = p * 2 + hh
            ji_a = 3 if ihalf == 0 else 11  # last ji touching OTp cols [0,512)
            # two independent 1-bank tiles: the [0,512) half drains (and its
            # slot frees for the NEXT head) right after ji_a instead of after
            # the whole ji loop.
            OTpA = psO.tile([128, 512], F32, tag="OTpA")
            OTpB = psO.tile([128, 512], F32, tag="OTpB")
            cp = denp.tile([128, 1024], F32, tag="cp")

            def den_half(c0):
                # normalize rows 0..63 by 1/rowsum (row 64), write bf16 OTsb
                nc.vector.tensor_copy(
                    out=cp[0:65, c0 : c0 + 512],
                    in_=(OTpA if c0 == 0 else OTpB)[0:65, :],
                )
                rec = denp.tile([1, 512], BF16, tag="rec")
                with nc.allow_low_precision(reason="1/rowsum bcast in bf16"):
                    nc.vector.reciprocal(out=rec, in_=cp[64:65, c0 : c0 + 512])
                dscr = drp.tile([1, 512], BF16, tag="dscr")
                nc.sync.dma_start(out=dscr, in_=rec)
                bc = denp.tile([64, 512], BF16, tag="bc")
                nc.sync.dma_start(out=bc, in_=dscr.broadcast_to((64, 512)))
                cpb = denp.tile([64, 512], BF16, tag="cpb")
                nc.vector.tensor_tensor(
                    out=cpb, in0=cp[0:64, c0 : c0 + 512], in1=bc,
                    op=mybir.AluOpType.mult,
                )
                nc.sync.dma_start(
                    out=OTsb[row0 : row0 + 64, p,
                             half0 + c0 : half0 + c0 + 512],
                    in_=cpb,
                )

            for ji in range(8 if ihalf == 0 else 16):
                dt_i = (ji * 128) // 512 * 512
                i_lo = max(half0, dt_i)
                W = half0 + 1024 - i_lo
                nblk = W // 512
                d = 0
                has_mask = dt_i >= half0
                if has_mask:
                    d = ji * 128 - dt_i
                Sp = psS.tile([128, 1024], F32, tag="Sp")
                for s5 in range(nblk):
                    lo = d if s5 == 0 else 0
                    nc.tensor.matmul(
                        Sp[:, s5 * 512 + lo : (s5 + 1) * 512],
                        lhsT=kT[row0 : row0 + 64, p, ji * 128 : (ji + 1) * 128],
                        rhs=qT[row0 : row0 + 64, p,
                               i_lo + s5 * 512 + lo : i_lo + (s5 + 1) * 512],
                        start=True, stop=True,
                    )
                expS = expp.tile([128, 1024], BF16, tag="expS")
                nc.scalar.activation(
                    out=expS[:, d:W], in_=Sp[:, d:W],
                    func=mybir.ActivationFunctionType.Exp,
                )
                if has_mask:
                    nc.vector.tensor_tensor(
                        out=expS[:, d : d + 128],
                        in0=expS[:, d : d + 128], in1=trisb,
                        op=mybir.AluOpType.mult,
                    )
                for s5 in range(nblk):
                    blk_i = i_lo + s5 * 512
                    off = blk_i - half0
                    lo = d if s5 == 0 else 0
                    OTp_t = OTpA if off == 0 else OTpB
                    nc.tensor.matmul(
                        OTp_t[0:65, lo : 512] if off == 0
                        else OTp_t[0:65, off + lo - 512 : off - 512 + 512],
                        lhsT=vsc[:, ji, h, :],
                        rhs=expS[:, s5 * 512 + lo : (s5 + 1) * 512],
                        start=(ji == 0), stop=(ji == blk_i // 128 + 3),
                    )
                if ji == ji_a:
                    # cols [0,512) are final: normalize + drain that PSUM
                    # bank early so the psO slot half frees and the OTsb
                    # consumer (Wo) is unblocked sooner.
                    den_half(0)
                if fill:
                    fill.pop(0)()
            den_half(512)

        # ---------------- schedule ---------------------------------------
        def spread(items, nslots):
            """Pad `items` to nslots with no-ops, spacing them evenly."""
            out = [nothing] * nslots
            n = len(items)
            assert n <= nslots
            for i, it in enumerate(items):
                out[i * nslots // n] = it
            return out

        nothing = lambda: None

        with ExitStack() as attn_es:
            expp = attn_es.enter_context(tc.tile_pool(name="expp", bufs=3))
            denp = attn_es.enter_context(tc.tile_pool(name="denp", bufs=3))
            drp = attn_es.enter_context(
                tc.tile_pool(name="drp", bufs=2, space="DRAM")
            )
            psS = attn_es.enter_context(
                tc.tile_pool(name="psS", bufs=2, space="PSUM")
            )
            psO = attn_es.enter_context(
                tc.tile_pool(name="psO", bufs=1, space="PSUM")
            )

            with ExitStack() as w23:
                psB = w23.enter_context(
                    tc.tile_pool(name="psB", bufs=2, space="PSUM")
                )

                # Windows 1+2 merged: LN half 0 pipelined with v/qk-proj so
                # PE starts as soon as the first transposed tiles land. DMA
                # order matters: the x tile gating the first transpose goes
                # first on the (serialized) DMA pipe, weights fill the rest.
                ln_tile(0)
                ln_tile(1)
                load_wv()
                vproj(0, psB)
                ln_tile(2)
                vproj(1, psB)
                load_wqk()
                ln_tile(3)
                vproj(2, psB)
                qkproj(0, 0, psB)
                ln_tile(4)
                vproj(3, psB)
                qkproj(1, 0, psB)
                load_wo()
                ln_tile(5)
                vproj(4, psB)
                ln_tile(6)
                vproj(5, psB)
                ln_tile(7)
                vproj(6, psB)
                vproj(7, psB)
                qkproj(0, 1, psB)
                qkproj(1, 1, psB)

                # Window 3: attention over the i<1024 triangle, interleaved
                # with the rest of half-0 q/k-proj (ordered so head (0,p,*)
                # finds its qk done) and LN + v/qk-proj of token half 1.
                work = []
                for t4 in range(2):
                    for p in range(2, 4):
                        work.append(lambda p=p, t4=t4: qkproj(p, t4, psB))
                for tt in range(8, 16):
                    work.append(lambda tt=tt: ln_tile(tt))
                    if tt >= 9:
                        work.append(lambda tt=tt - 1: vproj(tt, psB))
                work.append(lambda: vproj(15, psB))
                for t4 in range(2, 4):
                    for p in range(4):
                        work.append(lambda p=p, t4=t4: qkproj(p, t4, psB))

                # 8 heads x 8 ji slots = 64 fill slots.
                fills = spread(work, 64)
                for p in range(4):
                    for hh in range(2):
                        k = (p * 2 + hh) * 8
                        attn_head(0, p, hh, fill=list(fills[k : k + 8]))

            # Window 4: attention for i>=1024 (full j range), Wo(half 0)
            # streamed between heads.
            with ExitStack() as w4:
                psD = w4.enter_context(
                    tc.tile_pool(name="psD", bufs=2, space="PSUM")
                )
                wo0 = []
                for tt in range(8):
                    for e2 in range(2):
                        wo0.append(lambda tt=tt, e2=e2: wo_tile(tt, e2, psD))
                fill_iter = iter(wo0)
                for p in range(4):
                    for hh in range(2):
                        slots = []
                        for _ in range(2):
                            slots.append(next(fill_iter, nothing))
                        attn_head(1, p, hh, fill=slots + [nothing] * 14)
                for item in fill_iter:
                    item()

        # Tail: Wo(half 1) on a deep PSUM pipeline.
        psD2 = ctx.enter_context(tc.tile_pool(name="psD2", bufs=6, space="PSUM"))
        for tt in range(8, 16):
            for e2 in range(2):
                wo_tile(tt, e2, psD2)


_CACHE = {}


def _get_module(repeat: int = 1):
    if repeat not in _CACHE:
        _CACHE[repeat] = build_module(repeat)
    return _CACHE[repeat]


def _make_tri():
    r = np.arange(128)[:, None]
    c = np.arange(128)[None, :]
    return (c >= r).astype(ml_dtypes.bfloat16)  # 1 = attend (j <= i)


def _prep_in_maps(x, g_ln, Wq, Wkv, Wo):
    x = np.asarray(x, dtype=np.float32)
    g_ln = np.asarray(g_ln, dtype=np.float32)
    Wq = np.asarray(Wq, dtype=np.float32)
    Wkv = np.asarray(Wkv, dtype=np.float32)
    Wo = np.asarray(Wo, dtype=np.float32)

    scale = np.float32(DH ** -0.5)
    wq_full = (g_ln[:, None] * Wq * scale).astype(ml_dtypes.bfloat16)
    wk_full = (g_ln[:, None] * Wkv[:, :D]).astype(ml_dtypes.bfloat16)
    wv_full = (g_ln[:, None] * Wkv[:, D:]).astype(ml_dtypes.bfloat16)
    wo_bf = Wo.astype(ml_dtypes.bfloat16)
    x_bf = x.astype(ml_dtypes.bfloat16)

    def relay(w):  # [1024, m] -> [128, 8, m] with d = dk*128 + r
        return np.ascontiguousarray(w.reshape(8, 128, -1).transpose(1, 0, 2))

    def relay_o(w):  # [512, 1024] -> [128, 4, 1024] with c = ck*128 + r
        return np.ascontiguousarray(w.reshape(4, 128, -1).transpose(1, 0, 2))

    tri = _make_tri()

    in_maps = []
    for c in range(N_CORES):
        b, g = c // 2, c % 2
        sl = slice(g * 512, (g + 1) * 512)
        in_maps.append(
            {
                "x": np.ascontiguousarray(x_bf[b]),
                "wq": relay(wq_full[:, sl]),
                "wk": relay(wk_full[:, sl]),
                "wv": relay(wv_full[:, sl]),
                "wo": relay_o(wo_bf[sl, :]),
                "tri": tri,
            }
        )
    return in_maps


def kernel(x, g_ln, Wq, Wkv, Wo):
    nc = _get_module(repeat=1)
    in_maps = _prep_in_maps(x, g_ln, Wq, Wkv, Wo)
    res = run_bass_kernel_spmd(nc, in_maps, list(range(N_CORES)))
    out = np.empty((B, N, D), dtype=np.float32)
    for b in range(B):
        out[b] = res.results[2 * b]["out"].astype(np.float32) + res.results[
            2 * b + 1
        ]["out"].astype(np.float32)
    return out


# revision 36
# speedup vs baseline: 1.8695x; 1.2516x over previous
"""Fused LayerNorm + causal multi-head attention + output projection for
Trainium2, distributed over 8 NeuronCores.

Problem (full shapes): x [4, 2048, 1024], g_ln [1024], Wq [1024, 1024],
Wkv [1024, 2048], Wo [1024, 1024]; B=4, N=2048, D=1024, H=16, DH=64.

Sharding: DP(batch)=4 x TP(heads)=2. Core c handles batch b=c//2 and head
group g=c%2 (heads [g*8, g*8+8)). Each core computes LN(x_b), projects
q/k/v for its 8 heads (g_ln and the 1/sqrt(DH) scale are folded into the
weights host-side), runs causal attention, and multiplies by its slice of
Wo rows, producing a partial [2048, 1024] output. The host sums the two
partials per batch (row-parallel Wo reduce done on host).

v2 design notes (vs the fp32r v1):
 - All activations/weights are bf16 on chip (matmuls run at the same
   1 cycle/row as fp32r, but transposes are 2x faster and SBUF/DMA
   traffic halves). PSUM accumulation stays fp32.
 - xn^T is produced by the DMA XBAR transpose (16-bit only), freeing
   both the PE (no transpose matmuls) and ACT (no PSUM->SBUF copies).
 - Scores are computed transposed (S^T[j, i]) so softmax denominators
   come from a ones-column appended to V; no P transposes needed.
 - Causal masking multiplies the post-exp diagonal [128,128] block by a
   binary lower-triangle; fully-masked blocks are skipped (trimmed
   QK/exp/PV ranges).
 - Emission order software-pipelines the phases: LN/proj of the second
   token half and the i<1024 attention triangle interleave, and the
   Wo projection of the first half streams inside the i>=1024 attention
   window, keeping PE busy while ACT grinds exp().
"""

import sys

for _p in ("/opt/trn_rl_repo",):
    if _p not in sys.path:
        sys.path.insert(0, _p)

import numpy as np
import ml_dtypes

import concourse.bacc as bacc
import concourse.mybir as mybir
import concourse.tile as tile
from concourse.bass_utils import run_bass_kernel_spmd

# Route every ACT function this kernel uses (Ln, Exp, Copy) to the single
# table set that contains them all, so insert_act_table_loads emits exactly
# one ACT_TABLE_LOAD instead of thrashing between per-function sets (the
# pass picks the first matching set otherwise: Exp->exp_and_others,
# Ln->natural_log => a 1.3us reload per LN tile interleaved with attention
# exp). Indices must stay stable (walrus keys act.json by list position),
# so the other entries are kept but emptied rather than removed.
import concourse.hw_specs as _hw_specs
import concourse.bacc as _bacc_mod
import functools as _functools

_orig_get_act_tables = _hw_specs.get_activation_tables


@_functools.cache
def _patched_get_act_tables(arch):
    tabs = dict(_orig_get_act_tables(arch))
    keep = "natural_log_exp_and_others"
    assert keep in tabs
    need = {
        mybir.ActivationFunctionType.Ln,
        mybir.ActivationFunctionType.Exp,
        mybir.ActivationFunctionType.Copy,
        mybir.ActivationFunctionType.Identity,
    }
    assert need <= tabs[keep], (need, tabs[keep])
    return {name: (fs if name == keep else set()) for name, fs in tabs.items()}


_hw_specs.get_activation_tables = _patched_get_act_tables
_bacc_mod.get_activation_tables = _patched_get_act_tables
import concourse.bass_interp as _bass_interp_mod

_bass_interp_mod.get_activation_tables = _patched_get_act_tables

N_CORES = 8
B, N, D, H = 4, 2048, 1024, 16
DH = D // H
HL = 8  # heads per core
EPS = 1e-5
F32 = mybir.dt.float32
BF16 = mybir.dt.bfloat16


def build_module(repeat: int = 1):
    nc = bacc.Bacc("TRN2", target_bir_lowering=False)

    x_h = nc.dram_tensor("x", [N, D], BF16, kind="ExternalInput")
    # weights come host-prelayouted as [r, dk, m] so the DMA reads one
    # contiguous 8KB run per partition (128 descriptors, not 1024)
    wq_h = nc.dram_tensor("wq", [128, 8, 512], BF16, kind="ExternalInput")
    wk_h = nc.dram_tensor("wk", [128, 8, 512], BF16, kind="ExternalInput")
    wv_h = nc.dram_tensor("wv", [128, 8, 512], BF16, kind="ExternalInput")
    wo_h = nc.dram_tensor("wo", [128, 4, D], BF16, kind="ExternalInput")
    tri_h = nc.dram_tensor("tri", [128, 128], BF16, kind="ExternalInput")
    out_h = nc.dram_tensor("out", [N, D], BF16, kind="ExternalOutput")

    with tile.TileContext(nc) as tc:

        def body(_iv=None):
            _body(nc, tc, x_h, wq_h, wk_h, wv_h, wo_h, tri_h, out_h)

        if repeat == 1:
            body()
        else:
            with tc.For_i(0, repeat, 1):
                body()

    nc.compile()
    return nc


def _body(nc, tc, x_h, wq_h, wk_h, wv_h, wo_h, tri_h, out_h):
    from contextlib import ExitStack

    with ExitStack() as ctx:
        persist = ctx.enter_context(tc.tile_pool(name="persist", bufs=1))

        trisb = persist.tile([128, 128], BF16)
        nc.sync.dma_start(out=trisb, in_=tri_h[:, :])

        xnT = persist.tile([128, 8, N], BF16)
        qT = persist.tile([128, 4, N], BF16)
        kT = persist.tile([128, 4, N], BF16)
        vsc = persist.tile([128, 16, HL, 65], BF16)
        OTsb = persist.tile([128, 4, N], BF16)

        wv_sb = persist.tile([128, 8, 512], BF16)
        wo_sb = persist.tile([128, 4, D], BF16)
        wq_sb = persist.tile([128, 8, 512], BF16)
        wk_sb = persist.tile([128, 8, 512], BF16)

        abp = ctx.enter_context(tc.tile_pool(name="abp", bufs=1))
        lnp = ctx.enter_context(tc.tile_pool(name="lnp", bufs=3))
        wsp = ctx.enter_context(tc.tile_pool(name="wsp", bufs=2))

        eps_t = abp.tile([128, 1], F32)
        nc.vector.memset(eps_t, EPS)
        ones8 = abp.tile([128, 8], BF16)
        nc.vector.memset(ones8, 1.0)

        # ---------------- helpers ----------------------------------------
        def ln_tile(tt):
            """LN token tile tt -> xnT[:, :, tt*128:(tt+1)*128] (bf16)."""
            t0 = tt * 128
            xt = lnp.tile([128, D], BF16, tag="xt", bufs=4)
            nc.sync.dma_start(out=xt, in_=x_h[t0 : t0 + 128, :])
            st = lnp.tile([128, 2, 6], F32, tag="st")
            for sg in range(2):
                nc.vector.bn_stats(
                    out=st[:, sg, :], in_=xt[:, sg * 512 : (sg + 1) * 512]
                )
            mv = lnp.tile([128, 2], F32, tag="mv")
            nc.vector.bn_aggr(out=mv, in_=st)
            # rsqrt(var+eps) = exp(-0.5*ln(var+eps)); ln+exp live in one ACT
            # table set (natural_log_exp_and_others) so interleaving with the
            # attention exp() does not thrash ACT_TABLE_LOAD.
            lg = lnp.tile([128, 1], F32, tag="lg")
            nc.scalar.activation(
                out=lg, in_=mv[:, 1:2],
                func=mybir.ActivationFunctionType.Ln,
                bias=eps_t, scale=1.0,
            )
            rs = lnp.tile([128, 1], F32, tag="rs")
            nc.scalar.activation(
                out=rs, in_=lg,
                func=mybir.ActivationFunctionType.Exp,
                scale=-0.5,
            )
            xtn = lnp.tile([128, D], BF16, tag="xtn", bufs=3)
            nc.vector.tensor_scalar(
                out=xtn, in0=xt, scalar1=mv[:, 0:1], scalar2=rs,
                op0=mybir.AluOpType.subtract, op1=mybir.AluOpType.mult,
            )
            # XBAR transpose: [128 tok, 1024 d] -> [128 r, 8 dk, 128 tok]
            nc.sync.dma_start(
                out=xnT[:, :, t0 : t0 + 128], in_=xtn, transpose=True
            )

        def load_wv():
            nc.sync.dma_start(out=wv_sb, in_=wv_h[:, :, :])

        def load_wo():
            nc.sync.dma_start(out=wo_sb, in_=wo_h[:, :, :])

        def load_wqk():
            nc.sync.dma_start(out=wq_sb, in_=wq_h[:, :, :])
            nc.sync.dma_start(out=wk_sb, in_=wk_h[:, :, :])

        def vproj(tt, pool):
            """v projection for token tile tt -> vsc[:, tt, :, :]."""
            psv = pool.tile([128, 512], F32, tag="pp")
            for dk in range(8):
                nc.tensor.matmul(
                    psv, lhsT=xnT[:, dk, tt * 128 : (tt + 1) * 128],
                    rhs=wv_sb[:, dk, :],
                    start=(dk == 0), stop=(dk == 7),
                )
            nc.vector.tensor_copy(
                out=vsc[:, tt, :, 0:64],
                in_=psv.rearrange("r (h d) -> r h d", h=HL),
            )
            nc.vector.tensor_copy(
                out=vsc[:, tt, :, 64:65].rearrange("p h o -> p (h o)"),
                in_=ones8,
            )

        def qkproj(p, t4, pool):
            """q/k projection for dim block p, token half-quarter t4 (512)."""
            p0 = p * 128
            tok0 = t4 * 512
            psq = pool.tile([128, 512], F32, tag="pp")
            for dk in range(8):
                nc.tensor.matmul(
                    psq, lhsT=wq_sb[:, dk, p0 : p0 + 128],
                    rhs=xnT[:, dk, tok0 : tok0 + 512],
                    start=(dk == 0), stop=(dk == 7),
                )
            nc.vector.tensor_copy(out=qT[:, p, tok0 : tok0 + 512], in_=psq)
            psk = pool.tile([128, 512], F32, tag="pp")
            for dk in range(8):
                nc.tensor.matmul(
                    psk, lhsT=wk_sb[:, dk, p0 : p0 + 128],
                    rhs=xnT[:, dk, tok0 : tok0 + 512],
                    start=(dk == 0), stop=(dk == 7),
                )
            nc.vector.tensor_copy(out=kT[:, p, tok0 : tok0 + 512], in_=psk)

        def wo_tile(tt, e2, pool):
            pso = pool.tile([128, 512], F32, tag="pso")
            for ck in range(4):
                nc.tensor.matmul(
                    pso, lhsT=OTsb[:, ck, tt * 128 : (tt + 1) * 128],
                    rhs=wo_sb[:, ck, e2 * 512 : (e2 + 1) * 512],
                    start=(ck == 0), stop=(ck == 3),
                )
            osb = outp.tile([128, 512], BF16, tag="osb")
            nc.vector.tensor_copy(out=osb, in_=pso)
            nc.sync.dma_start(
                out=out_h[tt * 128 : (tt + 1) * 128, e2 * 512 : (e2 + 1) * 512],
                in_=osb,
            )

        outp = ctx.enter_context(tc.tile_pool(name="outp", bufs=4))

        def attn_head(ihalf, p, hh, fill=None):
            """Causal attention for head (p, hh), query half ihalf.

            fill: optional list of zero-arg closures; one is invoked after
            each ji iteration to interleave non-attention work into the
            emission stream (software pipelining across engines).
            """
            half0 = ihalf * 1024
            row0 = hh * 64
            h = p * 2 + hh
            ji_a = 3 if ihalf == 0 else 11  # last ji touching OTp cols [0,512)
            # two independent 1-bank tiles: the [0,512) half drains (and its
            # slot frees for the NEXT head) right after ji_a instead of after
            # the whole ji loop.
            OTpA = psO.tile([128, 512], F32, tag="OTpA")
            OTpB = psO.tile([128, 512], F32, tag="OTpB")
            cp = denp.tile([128, 1024], F32, tag="cp")

            def den_half(c0):
                # normalize rows 0..63 by 1/rowsum (row 64), write bf16 OTsb
                nc.vector.tensor_copy(
                    out=cp[0:65, c0 : c0 + 512],
                    in_=(OTpA if c0 == 0 else OTpB)[0:65, :],
                )
                rec = denp.tile([1, 512], BF16, tag="rec")
                with nc.allow_low_precision(reason="1/rowsum bcast in bf16"):
                    nc.vector.reciprocal(out=rec, in_=cp[64:65, c0 : c0 + 512])
                dscr = drp.tile([1, 512], BF16, tag="dscr")
                nc.sync.dma_start(out=dscr, in_=rec)
                bc = denp.tile([64, 512], BF16, tag="bc")
                nc.sync.dma_start(out=bc, in_=dscr.broadcast_to((64, 512)))
                cpb = denp.tile([64, 512], BF16, tag="cpb")
                nc.vector.tensor_tensor(
                    out=cpb, in0=cp[0:64, c0 : c0 + 512], in1=bc,
                    op=mybir.AluOpType.mult,
                )
                nc.sync.dma_start(
                    out=OTsb[row0 : row0 + 64, p,
                             half0 + c0 : half0 + c0 + 512],
                    in_=cpb,
                )

            nji = 8 if ihalf == 0 else 16

            def ji_params(ji):
                dt_i = (ji * 128) // 512 * 512
                i_lo = max(half0, dt_i)
                W = half0 + 1024 - i_lo
                d = ji * 128 - dt_i if dt_i >= half0 else 0
                return dt_i, i_lo, W, W // 512, d

            def emit_qk(ji):
                dt_i, i_lo, W, nblk, d = ji_params(ji)
                Sp = psS.tile([128, 1024], F32, tag="Sp")
                for s5 in range(nblk):
                    lo = d if s5 == 0 else 0
                    nc.tensor.matmul(
                        Sp[:, s5 * 512 + lo : (s5 + 1) * 512],
                        lhsT=kT[row0 : row0 + 64, p, ji * 128 : (ji + 1) * 128],
                        rhs=qT[row0 : row0 + 64, p,
                               i_lo + s5 * 512 + lo : i_lo + (s5 + 1) * 512],
                        start=True, stop=True,
                    )
                return Sp

            # Software-pipelined inner loop: QK(ji+1) is emitted BEFORE the
            # exp(ji)-dependent PV(ji) so the in-order PE queue never blocks
            # on ACT; fill work slots in behind it.
            Sp_cur = emit_qk(0)
            for ji in range(nji):
                dt_i, i_lo, W, nblk, d = ji_params(ji)
                has_mask = dt_i >= half0
                Sp = Sp_cur
                if ji + 1 < nji:
                    Sp_cur = emit_qk(ji + 1)
                expS = expp.tile([128, 1024], BF16, tag="expS")
                nc.scalar.activation(
                    out=expS[:, d:W], in_=Sp[:, d:W],
                    func=mybir.ActivationFunctionType.Exp,
                )
                if has_mask:
                    nc.vector.tensor_tensor(
                        out=expS[:, d : d + 128],
                        in0=expS[:, d : d + 128], in1=trisb,
                        op=mybir.AluOpType.mult,
                    )
                if fill:
                    fill.pop(0)()
                for s5 in range(nblk):
                    blk_i = i_lo + s5 * 512
                    off = blk_i - half0
                    lo = d if s5 == 0 else 0
                    OTp_t = OTpA if off == 0 else OTpB
                    nc.tensor.matmul(
                        OTp_t[0:65, lo : 512] if off == 0
                        else OTp_t[0:65, off + lo - 512 : off - 512 + 512],
                        lhsT=vsc[:, ji, h, :],
                        rhs=expS[:, s5 * 512 + lo : (s5 + 1) * 512],
                        start=(ji == 0), stop=(ji == blk_i // 128 + 3),
                    )
                if ji == ji_a:
                    # cols [0,512) are final: normalize + drain that PSUM
                    # bank early so the psO slot half frees and the OTsb
                    # consumer (Wo) is unblocked sooner.
                    den_half(0)
            den_half(512)

        # ---------------- schedule ---------------------------------------
        def spread(items, nslots):
            """Pad `items` to nslots with no-ops, spacing them evenly."""
            out = [nothing] * nslots
            n = len(items)
            assert n <= nslots
            for i, it in enumerate(items):
                out[i * nslots // n] = it
            return out

        nothing = lambda: None

        with ExitStack() as attn_es:
            expp = attn_es.enter_context(tc.tile_pool(name="expp", bufs=3))
            denp = attn_es.enter_context(tc.tile_pool(name="denp", bufs=3))
            drp = attn_es.enter_context(
                tc.tile_pool(name="drp", bufs=2, space="DRAM")
            )
            psS = attn_es.enter_context(
                tc.tile_pool(name="psS", bufs=2, space="PSUM")
            )
            psO = attn_es.enter_context(
                tc.tile_pool(name="psO", bufs=1, space="PSUM")
            )

            with ExitStack() as w23:
                psB = w23.enter_context(
                    tc.tile_pool(name="psB", bufs=2, space="PSUM")
                )

                # Windows 1+2 merged: LN half 0 pipelined with v/qk-proj so
                # PE starts as soon as the first transposed tiles land. DMA
                # order matters: the x tile gating the first transpose goes
                # first on the (serialized) DMA pipe, weights fill the rest.
                ln_tile(0)
                ln_tile(1)
                load_wv()
                vproj(0, psB)
                ln_tile(2)
                vproj(1, psB)
                load_wqk()
                ln_tile(3)
                vproj(2, psB)
                qkproj(0, 0, psB)
                ln_tile(4)
                vproj(3, psB)
                qkproj(1, 0, psB)
                load_wo()
                ln_tile(5)
                vproj(4, psB)
                ln_tile(6)
                vproj(5, psB)
                ln_tile(7)
                vproj(6, psB)
                vproj(7, psB)
                qkproj(0, 1, psB)
                qkproj(1, 1, psB)

                # Window 3: attention over the i<1024 triangle, interleaved
                # with the rest of half-0 q/k-proj (ordered so head (0,p,*)
                # finds its qk done) and LN + v/qk-proj of token half 1.
                work = []
                for t4 in range(2):
                    for p in range(2, 4):
                        work.append(lambda p=p, t4=t4: qkproj(p, t4, psB))
                for tt in range(8, 16):
                    work.append(lambda tt=tt: ln_tile(tt))
                    if tt >= 9:
                        work.append(lambda tt=tt - 1: vproj(tt, psB))
                work.append(lambda: vproj(15, psB))
                for t4 in range(2, 4):
                    for p in range(4):
                        work.append(lambda p=p, t4=t4: qkproj(p, t4, psB))

                # 8 heads x 8 ji slots = 64 fill slots.
                fills = spread(work, 64)
                for p in range(4):
                    for hh in range(2):
                        k = (p * 2 + hh) * 8
                        attn_head(0, p, hh, fill=list(fills[k : k + 8]))

            # Window 4: attention for i>=1024 (full j range), Wo(half 0)
            # streamed between heads.
            with ExitStack() as w4:
                psD = w4.enter_context(
                    tc.tile_pool(name="psD", bufs=2, space="PSUM")
                )
                wo0 = []
                for tt in range(8):
                    for e2 in range(2):
                        wo0.append(lambda tt=tt, e2=e2: wo_tile(tt, e2, psD))
                fill_iter = iter(wo0)
                for p in range(4):
                    for hh in range(2):
                        slots = []
                        for _ in range(2):
                            slots.append(next(fill_iter, nothing))
                        attn_head(1, p, hh, fill=slots + [nothing] * 14)
                for item in fill_iter:
                    item()

        # Tail: Wo(half 1) on a deep PSUM pipeline.
        psD2 = ctx.enter_context(tc.tile_pool(name="psD2", bufs=6, space="PSUM"))
        for tt in range(8, 16):
            for e2 in range(2):
                wo_tile(tt, e2, psD2)


_CACHE = {}


def _get_module(repeat: int = 1):
    if repeat not in _CACHE:
        _CACHE[repeat] = build_module(repeat)
    return _CACHE[repeat]


def _make_tri():
    r = np.arange(128)[:, None]
    c = np.arange(128)[None, :]
    return (c >= r).astype(ml_dtypes.bfloat16)  # 1 = attend (j <= i)


def _prep_in_maps(x, g_ln, Wq, Wkv, Wo):
    x = np.asarray(x, dtype=np.float32)
    g_ln = np.asarray(g_ln, dtype=np.float32)
    Wq = np.asarray(Wq, dtype=np.float32)
    Wkv = np.asarray(Wkv, dtype=np.float32)
    Wo = np.asarray(Wo, dtype=np.float32)

    scale = np.float32(DH ** -0.5)
    wq_full = (g_ln[:, None] * Wq * scale).astype(ml_dtypes.bfloat16)
    wk_full = (g_ln[:, None] * Wkv[:, :D]).astype(ml_dtypes.bfloat16)
    wv_full = (g_ln[:, None] * Wkv[:, D:]).astype(ml_dtypes.bfloat16)
    wo_bf = Wo.astype(ml_dtypes.bfloat16)
    x_bf = x.astype(ml_dtypes.bfloat16)

    def relay(w):  # [1024, m] -> [128, 8, m] with d = dk*128 + r
        return np.ascontiguousarray(w.reshape(8, 128, -1).transpose(1, 0, 2))

    def relay_o(w):  # [512, 1024] -> [128, 4, 1024] with c = ck*128 + r
        return np.ascontiguousarray(w.reshape(4, 128, -1).transpose(1, 0, 2))

    tri = _make_tri()

    in_maps = []
    for c in range(N_CORES):
        b, g = c // 2, c % 2
        sl = slice(g * 512, (g + 1) * 512)
        in_maps.append(
            {
                "x": np.ascontiguousarray(x_bf[b]),
                "wq": relay(wq_full[:, sl]),
                "wk": relay(wk_full[:, sl]),
                "wv": relay(wv_full[:, sl]),
                "wo": relay_o(wo_bf[sl, :]),
                "tri": tri,
            }
        )
    return in_maps


def kernel(x, g_ln, Wq, Wkv, Wo):
    nc = _get_module(repeat=1)
    in_maps = _prep_in_maps(x, g_ln, Wq, Wkv, Wo)
    res = run_bass_kernel_spmd(nc, in_maps, list(range(N_CORES)))
    out = np.empty((B, N, D), dtype=np.float32)
    for b in range(B):
        out[b] = res.results[2 * b]["out"].astype(np.float32) + res.results[
            2 * b + 1
        ]["out"].astype(np.float32)
    return out


# revision 38
# speedup vs baseline: 1.9493x; 1.0427x over previous
"""Fused LayerNorm + causal multi-head attention + output projection for
Trainium2, distributed over 8 NeuronCores.

Problem (full shapes): x [4, 2048, 1024], g_ln [1024], Wq [1024, 1024],
Wkv [1024, 2048], Wo [1024, 1024]; B=4, N=2048, D=1024, H=16, DH=64.

Sharding: DP(batch)=4 x TP(heads)=2. Core c handles batch b=c//2 and head
group g=c%2 (heads [g*8, g*8+8)). Each core computes LN(x_b), projects
q/k/v for its 8 heads (g_ln and the 1/sqrt(DH) scale are folded into the
weights host-side), runs causal attention, and multiplies by its slice of
Wo rows, producing a partial [2048, 1024] output. The host sums the two
partials per batch (row-parallel Wo reduce done on host).

v2 design notes (vs the fp32r v1):
 - All activations/weights are bf16 on chip (matmuls run at the same
   1 cycle/row as fp32r, but transposes are 2x faster and SBUF/DMA
   traffic halves). PSUM accumulation stays fp32.
 - xn^T is produced by the DMA XBAR transpose (16-bit only), freeing
   both the PE (no transpose matmuls) and ACT (no PSUM->SBUF copies).
 - Scores are computed transposed (S^T[j, i]) so softmax denominators
   come from a ones-column appended to V; no P transposes needed.
 - Causal masking multiplies the post-exp diagonal [128,128] block by a
   binary lower-triangle; fully-masked blocks are skipped (trimmed
   QK/exp/PV ranges).
 - Emission order software-pipelines the phases: LN/proj of the second
   token half and the i<1024 attention triangle interleave, and the
   Wo projection of the first half streams inside the i>=1024 attention
   window, keeping PE busy while ACT grinds exp().
"""

import sys

for _p in ("/opt/trn_rl_repo",):
    if _p not in sys.path:
        sys.path.insert(0, _p)

import numpy as np
import ml_dtypes

import concourse.bacc as bacc
import concourse.mybir as mybir
import concourse.tile as tile
from concourse.bass_utils import run_bass_kernel_spmd

# Route every ACT function this kernel uses (Ln, Exp, Copy) to the single
# table set that contains them all, so insert_act_table_loads emits exactly
# one ACT_TABLE_LOAD instead of thrashing between per-function sets (the
# pass picks the first matching set otherwise: Exp->exp_and_others,
# Ln->natural_log => a 1.3us reload per LN tile interleaved with attention
# exp). Indices must stay stable (walrus keys act.json by list position),
# so the other entries are kept but emptied rather than removed.
import concourse.hw_specs as _hw_specs
import concourse.bacc as _bacc_mod
import functools as _functools

_orig_get_act_tables = _hw_specs.get_activation_tables


@_functools.cache
def _patched_get_act_tables(arch):
    tabs = dict(_orig_get_act_tables(arch))
    keep = "natural_log_exp_and_others"
    assert keep in tabs
    need = {
        mybir.ActivationFunctionType.Ln,
        mybir.ActivationFunctionType.Exp,
        mybir.ActivationFunctionType.Copy,
        mybir.ActivationFunctionType.Identity,
    }
    assert need <= tabs[keep], (need, tabs[keep])
    return {name: (fs if name == keep else set()) for name, fs in tabs.items()}


_hw_specs.get_activation_tables = _patched_get_act_tables
_bacc_mod.get_activation_tables = _patched_get_act_tables
import concourse.bass_interp as _bass_interp_mod

_bass_interp_mod.get_activation_tables = _patched_get_act_tables

N_CORES = 8
B, N, D, H = 4, 2048, 1024, 16
DH = D // H
HL = 8  # heads per core
EPS = 1e-5
F32 = mybir.dt.float32
BF16 = mybir.dt.bfloat16


def build_module(repeat: int = 1):
    nc = bacc.Bacc("TRN2", target_bir_lowering=False)

    x_h = nc.dram_tensor("x", [N, D], BF16, kind="ExternalInput")
    # weights come host-prelayouted as [r, dk, m] so the DMA reads one
    # contiguous 8KB run per partition (128 descriptors, not 1024)
    wq_h = nc.dram_tensor("wq", [128, 8, 512], BF16, kind="ExternalInput")
    wk_h = nc.dram_tensor("wk", [128, 8, 512], BF16, kind="ExternalInput")
    wv_h = nc.dram_tensor("wv", [128, 8, 512], BF16, kind="ExternalInput")
    wo_h = nc.dram_tensor("wo", [128, 4, D], BF16, kind="ExternalInput")
    tri_h = nc.dram_tensor("tri", [128, 128], BF16, kind="ExternalInput")
    out_h = nc.dram_tensor("out", [N, D], BF16, kind="ExternalOutput")

    with tile.TileContext(nc) as tc:

        def body(_iv=None):
            _body(nc, tc, x_h, wq_h, wk_h, wv_h, wo_h, tri_h, out_h)

        if repeat == 1:
            body()
        else:
            with tc.For_i(0, repeat, 1):
                body()

    nc.compile()
    return nc


def _body(nc, tc, x_h, wq_h, wk_h, wv_h, wo_h, tri_h, out_h):
    from contextlib import ExitStack

    with ExitStack() as ctx:
        persist = ctx.enter_context(tc.tile_pool(name="persist", bufs=1))

        trisb = persist.tile([128, 128], BF16)
        nc.sync.dma_start(out=trisb, in_=tri_h[:, :])

        xnT = persist.tile([128, 8, N], BF16)
        qT = persist.tile([128, 4, N], BF16)
        kT = persist.tile([128, 4, N], BF16)
        vsc = persist.tile([128, 16, HL, 65], BF16)
        OTsb = persist.tile([128, 4, N], BF16)

        wv_sb = persist.tile([128, 8, 512], BF16)
        wo_sb = persist.tile([128, 4, D], BF16)
        wq_sb = persist.tile([128, 8, 512], BF16)
        wk_sb = persist.tile([128, 8, 512], BF16)

        abp = ctx.enter_context(tc.tile_pool(name="abp", bufs=1))
        lnp = ctx.enter_context(tc.tile_pool(name="lnp", bufs=3))
        wsp = ctx.enter_context(tc.tile_pool(name="wsp", bufs=2))

        eps_t = abp.tile([128, 1], F32)
        nc.vector.memset(eps_t, EPS)
        ones8 = abp.tile([128, 8], BF16)
        nc.vector.memset(ones8, 1.0)

        # ---------------- helpers ----------------------------------------
        def ln_tile(tt):
            """LN token tile tt -> xnT[:, :, tt*128:(tt+1)*128] (bf16)."""
            t0 = tt * 128
            xt = lnp.tile([128, D], BF16, tag="xt", bufs=4)
            nc.sync.dma_start(out=xt, in_=x_h[t0 : t0 + 128, :])
            st = lnp.tile([128, 2, 6], F32, tag="st")
            for sg in range(2):
                nc.vector.bn_stats(
                    out=st[:, sg, :], in_=xt[:, sg * 512 : (sg + 1) * 512]
                )
            mv = lnp.tile([128, 2], F32, tag="mv")
            nc.vector.bn_aggr(out=mv, in_=st)
            # rsqrt(var+eps) = exp(-0.5*ln(var+eps)); ln+exp live in one ACT
            # table set (natural_log_exp_and_others) so interleaving with the
            # attention exp() does not thrash ACT_TABLE_LOAD.
            lg = lnp.tile([128, 1], F32, tag="lg")
            nc.scalar.activation(
                out=lg, in_=mv[:, 1:2],
                func=mybir.ActivationFunctionType.Ln,
                bias=eps_t, scale=1.0,
            )
            rs = lnp.tile([128, 1], F32, tag="rs")
            nc.scalar.activation(
                out=rs, in_=lg,
                func=mybir.ActivationFunctionType.Exp,
                scale=-0.5,
            )
            xtn = lnp.tile([128, D], BF16, tag="xtn", bufs=3)
            nc.vector.tensor_scalar(
                out=xtn, in0=xt, scalar1=mv[:, 0:1], scalar2=rs,
                op0=mybir.AluOpType.subtract, op1=mybir.AluOpType.mult,
            )
            # XBAR transpose: [128 tok, 1024 d] -> [128 r, 8 dk, 128 tok]
            nc.sync.dma_start(
                out=xnT[:, :, t0 : t0 + 128], in_=xtn, transpose=True
            )

        def load_wv():
            nc.sync.dma_start(out=wv_sb, in_=wv_h[:, :, :])

        def load_wo():
            nc.sync.dma_start(out=wo_sb, in_=wo_h[:, :, :])

        def load_wqk():
            nc.sync.dma_start(out=wq_sb, in_=wq_h[:, :, :])
            nc.sync.dma_start(out=wk_sb, in_=wk_h[:, :, :])

        def vproj(tt, pool):
            """v projection for token tile tt -> vsc[:, tt, :, :]."""
            psv = pool.tile([128, 512], F32, tag="pp")
            for dk in range(8):
                nc.tensor.matmul(
                    psv, lhsT=xnT[:, dk, tt * 128 : (tt + 1) * 128],
                    rhs=wv_sb[:, dk, :],
                    start=(dk == 0), stop=(dk == 7),
                )
            nc.vector.tensor_copy(
                out=vsc[:, tt, :, 0:64],
                in_=psv.rearrange("r (h d) -> r h d", h=HL),
            )
            nc.vector.tensor_copy(
                out=vsc[:, tt, :, 64:65].rearrange("p h o -> p (h o)"),
                in_=ones8,
            )

        def qkproj(p, t4, pool):
            """q/k projection for dim block p, token half-quarter t4 (512)."""
            p0 = p * 128
            tok0 = t4 * 512
            psq = pool.tile([128, 512], F32, tag="pp")
            for dk in range(8):
                nc.tensor.matmul(
                    psq, lhsT=wq_sb[:, dk, p0 : p0 + 128],
                    rhs=xnT[:, dk, tok0 : tok0 + 512],
                    start=(dk == 0), stop=(dk == 7),
                )
            nc.vector.tensor_copy(out=qT[:, p, tok0 : tok0 + 512], in_=psq)
            psk = pool.tile([128, 512], F32, tag="pp")
            for dk in range(8):
                nc.tensor.matmul(
                    psk, lhsT=wk_sb[:, dk, p0 : p0 + 128],
                    rhs=xnT[:, dk, tok0 : tok0 + 512],
                    start=(dk == 0), stop=(dk == 7),
                )
            nc.vector.tensor_copy(out=kT[:, p, tok0 : tok0 + 512], in_=psk)

        def wo_tile(tt, e2, pool):
            pso = pool.tile([128, 512], F32, tag="pso")
            for ck in range(4):
                nc.tensor.matmul(
                    pso, lhsT=OTsb[:, ck, tt * 128 : (tt + 1) * 128],
                    rhs=wo_sb[:, ck, e2 * 512 : (e2 + 1) * 512],
                    start=(ck == 0), stop=(ck == 3),
                )
            osb = outp.tile([128, 512], BF16, tag="osb")
            nc.vector.tensor_copy(out=osb, in_=pso)
            nc.sync.dma_start(
                out=out_h[tt * 128 : (tt + 1) * 128, e2 * 512 : (e2 + 1) * 512],
                in_=osb,
            )

        outp = ctx.enter_context(tc.tile_pool(name="outp", bufs=4))

        def attn_head(ihalf, p, hh, fill=None):
            """Causal attention for head (p, hh), query half ihalf.

            fill: optional list of zero-arg closures; one is invoked after
            each ji iteration to interleave non-attention work into the
            emission stream (software pipelining across engines).
            """
            half0 = ihalf * 1024
            row0 = hh * 64
            h = p * 2 + hh
            ji_a = 3 if ihalf == 0 else 11  # last ji touching OTp cols [0,512)
            # two independent 1-bank tiles: the [0,512) half drains (and its
            # slot frees for the NEXT head) right after ji_a instead of after
            # the whole ji loop.
            OTpA = psO.tile([128, 512], F32, tag="OTpA")
            OTpB = psO.tile([128, 512], F32, tag="OTpB")
            cp = denp.tile([128, 1024], F32, tag="cp")

            def den_copy(c0):
                # drain one OTp PSUM bank; this is the only DVE op that gates
                # the next head's PV, so nothing else sits in front of it
                nc.vector.tensor_copy(
                    out=cp[0:65, c0 : c0 + 512],
                    in_=(OTpA if c0 == 0 else OTpB)[0:65, :],
                )

            def den_finish(c0):
                # normalize rows 0..63 by 1/rowsum (row 64), write bf16 OTsb
                rec = denp.tile([1, 512], BF16, tag="rec")
                with nc.allow_low_precision(reason="1/rowsum bcast in bf16"):
                    nc.vector.reciprocal(out=rec, in_=cp[64:65, c0 : c0 + 512])
                dscr = drp.tile([1, 512], BF16, tag="dscr")
                nc.sync.dma_start(out=dscr, in_=rec)
                bc = denp.tile([64, 512], BF16, tag="bc")
                nc.sync.dma_start(out=bc, in_=dscr.broadcast_to((64, 512)))
                cpb = denp.tile([64, 512], BF16, tag="cpb")
                nc.vector.tensor_tensor(
                    out=cpb, in0=cp[0:64, c0 : c0 + 512], in1=bc,
                    op=mybir.AluOpType.mult,
                )
                nc.sync.dma_start(
                    out=OTsb[row0 : row0 + 64, p,
                             half0 + c0 : half0 + c0 + 512],
                    in_=cpb,
                )

            nji = 8 if ihalf == 0 else 16

            def ji_params(ji):
                dt_i = (ji * 128) // 512 * 512
                i_lo = max(half0, dt_i)
                W = half0 + 1024 - i_lo
                d = ji * 128 - dt_i if dt_i >= half0 else 0
                return dt_i, i_lo, W, W // 512, d

            def emit_qk(ji):
                dt_i, i_lo, W, nblk, d = ji_params(ji)
                Sp = psS.tile([128, 1024], F32, tag="Sp")
                for s5 in range(nblk):
                    lo = d if s5 == 0 else 0
                    nc.tensor.matmul(
                        Sp[:, s5 * 512 + lo : (s5 + 1) * 512],
                        lhsT=kT[row0 : row0 + 64, p, ji * 128 : (ji + 1) * 128],
                        rhs=qT[row0 : row0 + 64, p,
                               i_lo + s5 * 512 + lo : i_lo + (s5 + 1) * 512],
                        start=True, stop=True,
                    )
                return Sp

            # Software-pipelined inner loop: QK(ji+1) is emitted BEFORE the
            # exp(ji)-dependent PV(ji) so the in-order PE queue never blocks
            # on ACT; fill work slots in behind it.
            Sp_cur = emit_qk(0)
            for ji in range(nji):
                dt_i, i_lo, W, nblk, d = ji_params(ji)
                has_mask = dt_i >= half0
                Sp = Sp_cur
                if ji + 1 < nji:
                    Sp_cur = emit_qk(ji + 1)
                expS = expp.tile([128, 1024], BF16, tag="expS")
                nc.scalar.activation(
                    out=expS[:, d:W], in_=Sp[:, d:W],
                    func=mybir.ActivationFunctionType.Exp,
                )
                if has_mask:
                    nc.vector.tensor_tensor(
                        out=expS[:, d : d + 128],
                        in0=expS[:, d : d + 128], in1=trisb,
                        op=mybir.AluOpType.mult,
                    )
                if fill:
                    fill.pop(0)()
                for s5 in range(nblk):
                    blk_i = i_lo + s5 * 512
                    off = blk_i - half0
                    lo = d if s5 == 0 else 0
                    OTp_t = OTpA if off == 0 else OTpB
                    nc.tensor.matmul(
                        OTp_t[0:65, lo : 512] if off == 0
                        else OTp_t[0:65, off + lo - 512 : off - 512 + 512],
                        lhsT=vsc[:, ji, h, :],
                        rhs=expS[:, s5 * 512 + lo : (s5 + 1) * 512],
                        start=(ji == 0), stop=(ji == blk_i // 128 + 3),
                    )
                if ji == ji_a:
                    # cols [0,512) are final: normalize + drain that PSUM
                    # bank early so the psO slot half frees and the OTsb
                    # consumer (Wo) is unblocked sooner.
                    den_copy(0)
            den_copy(512)
            den_finish(0)
            den_finish(512)

        # ---------------- schedule ---------------------------------------
        def spread(items, nslots):
            """Pad `items` to nslots with no-ops, spacing them evenly."""
            out = [nothing] * nslots
            n = len(items)
            assert n <= nslots
            for i, it in enumerate(items):
                out[i * nslots // n] = it
            return out

        nothing = lambda: None

        with ExitStack() as attn_es:
            expp = attn_es.enter_context(tc.tile_pool(name="expp", bufs=3))
            denp = attn_es.enter_context(tc.tile_pool(name="denp", bufs=3))
            drp = attn_es.enter_context(
                tc.tile_pool(name="drp", bufs=2, space="DRAM")
            )
            psS = attn_es.enter_context(
                tc.tile_pool(name="psS", bufs=2, space="PSUM")
            )
            psO = attn_es.enter_context(
                tc.tile_pool(name="psO", bufs=1, space="PSUM")
            )

            with ExitStack() as w23:
                psB = w23.enter_context(
                    tc.tile_pool(name="psB", bufs=2, space="PSUM")
                )

                # Windows 1+2 merged: LN half 0 pipelined with v/qk-proj so
                # PE starts as soon as the first transposed tiles land. DMA
                # order matters: the x tile gating the first transpose goes
                # first on the (serialized) DMA pipe, weights fill the rest.
                ln_tile(0)
                ln_tile(1)
                load_wv()
                vproj(0, psB)
                ln_tile(2)
                vproj(1, psB)
                load_wqk()
                ln_tile(3)
                vproj(2, psB)
                qkproj(0, 0, psB)
                ln_tile(4)
                vproj(3, psB)
                qkproj(1, 0, psB)
                load_wo()
                ln_tile(5)
                vproj(4, psB)
                ln_tile(6)
                vproj(5, psB)
                ln_tile(7)
                vproj(6, psB)
                vproj(7, psB)
                qkproj(0, 1, psB)
                qkproj(1, 1, psB)

                # Window 3: attention over the i<1024 triangle, interleaved
                # with the rest of half-0 q/k-proj (ordered so head (0,p,*)
                # finds its qk done) and LN + v/qk-proj of token half 1.
                work = []
                for t4 in range(2):
                    for p in range(2, 4):
                        work.append(lambda p=p, t4=t4: qkproj(p, t4, psB))
                for tt in range(8, 16):
                    work.append(lambda tt=tt: ln_tile(tt))
                    if tt >= 9:
                        work.append(lambda tt=tt - 1: vproj(tt, psB))
                work.append(lambda: vproj(15, psB))
                for t4 in range(2, 4):
                    for p in range(4):
                        work.append(lambda p=p, t4=t4: qkproj(p, t4, psB))

                # 8 heads x 8 ji slots = 64 fill slots.
                fills = spread(work, 64)
                for p in range(4):
                    for hh in range(2):
                        k = (p * 2 + hh) * 8
                        attn_head(0, p, hh, fill=list(fills[k : k + 8]))

            # Window 4: attention for i>=1024 (full j range), Wo(half 0)
            # streamed between heads.
            with ExitStack() as w4:
                psD = w4.enter_context(
                    tc.tile_pool(name="psD", bufs=2, space="PSUM")
                )
                wo0 = []
                for tt in range(8):
                    for e2 in range(2):
                        wo0.append(lambda tt=tt, e2=e2: wo_tile(tt, e2, psD))
                fill_iter = iter(wo0)
                for p in range(4):
                    for hh in range(2):
                        slots = []
                        for _ in range(2):
                            slots.append(next(fill_iter, nothing))
                        attn_head(1, p, hh, fill=slots + [nothing] * 14)
                for item in fill_iter:
                    item()

        # Tail: Wo(half 1) on a deep PSUM pipeline.
        psD2 = ctx.enter_context(tc.tile_pool(name="psD2", bufs=6, space="PSUM"))
        for tt in range(8, 16):
            for e2 in range(2):
                wo_tile(tt, e2, psD2)


_CACHE = {}


def _get_module(repeat: int = 1):
    if repeat not in _CACHE:
        _CACHE[repeat] = build_module(repeat)
    return _CACHE[repeat]


def _make_tri():
    r = np.arange(128)[:, None]
    c = np.arange(128)[None, :]
    return (c >= r).astype(ml_dtypes.bfloat16)  # 1 = attend (j <= i)


def _prep_in_maps(x, g_ln, Wq, Wkv, Wo):
    x = np.asarray(x, dtype=np.float32)
    g_ln = np.asarray(g_ln, dtype=np.float32)
    Wq = np.asarray(Wq, dtype=np.float32)
    Wkv = np.asarray(Wkv, dtype=np.float32)
    Wo = np.asarray(Wo, dtype=np.float32)

    scale = np.float32(DH ** -0.5)
    wq_full = (g_ln[:, None] * Wq * scale).astype(ml_dtypes.bfloat16)
    wk_full = (g_ln[:, None] * Wkv[:, :D]).astype(ml_dtypes.bfloat16)
    wv_full = (g_ln[:, None] * Wkv[:, D:]).astype(ml_dtypes.bfloat16)
    wo_bf = Wo.astype(ml_dtypes.bfloat16)
    x_bf = x.astype(ml_dtypes.bfloat16)

    def relay(w):  # [1024, m] -> [128, 8, m] with d = dk*128 + r
        return np.ascontiguousarray(w.reshape(8, 128, -1).transpose(1, 0, 2))

    def relay_o(w):  # [512, 1024] -> [128, 4, 1024] with c = ck*128 + r
        return np.ascontiguousarray(w.reshape(4, 128, -1).transpose(1, 0, 2))

    tri = _make_tri()

    in_maps = []
    for c in range(N_CORES):
        b, g = c // 2, c % 2
        sl = slice(g * 512, (g + 1) * 512)
        in_maps.append(
            {
                "x": np.ascontiguousarray(x_bf[b]),
                "wq": relay(wq_full[:, sl]),
                "wk": relay(wk_full[:, sl]),
                "wv": relay(wv_full[:, sl]),
                "wo": relay_o(wo_bf[sl, :]),
                "tri": tri,
            }
        )
    return in_maps


def kernel(x, g_ln, Wq, Wkv, Wo):
    nc = _get_module(repeat=1)
    in_maps = _prep_in_maps(x, g_ln, Wq, Wkv, Wo)
    res = run_bass_kernel_spmd(nc, in_maps, list(range(N_CORES)))
    out = np.empty((B, N, D), dtype=np.float32)
    for b in range(B):
        out[b] = res.results[2 * b]["out"].astype(np.float32) + res.results[
            2 * b + 1
        ]["out"].astype(np.float32)
    return out
